# revision 1
# baseline (speedup 1.0000x reference)
"""MoE GemmaMLP (top-2 of 8 experts + shared expert) on 8 trn2 NeuronCores.

Sharding: expert-parallel with load balancing.  The host computes top-2
routing from router_logits, chunks each expert's routed batches into pairs,
and packs the pairs into "weight-stream groups" of <=2 pairs sharing one
expert.  The seed-0 load (pairs per expert [5,2,6,4,4,6,4,3] = 34) packs
into exactly 16 two-pair groups + 2 one-pair groups, so every core gets the
SPMD-uniform slot config (2,2,1) = 5 pairs — the integer optimum — with the
group's expert weights passed per-slot as zero-copy views.  Inside a group
the expert weights stream from HBM exactly once while gate/up/down are
interleaved per i-tile (down contracts CHUNK i-tiles in PSUM scratch, then
accumulates into per-pair SBUF output tiles).  The shared expert is
tensor-parallel over the intermediate dim: core c owns columns
[c*512, (c+1)*512) of shared gate/up (matching rows of shared down), kept
resident in SBUF, and processes all 32 batches.  The host sums the per-core
partials and applies the routing weights to the (unweighted) expert rows.
DMAs alternate between the SP and ACT HWDGE rings — a single ring's issue
path is a hard HW bottleneck the cost model misses (2.08ms -> 1.19ms).

Layout trick: x is transposed on the host to xT [B, H, S] so that every
matmul runs transpose-free on the PE:
  gate/up:  g^T[i,s] = sum_h Wg[h,i] * xT[h,s]   (lhsT = Wg tile, rhs = xT)
  down:     out[s,h] = sum_i a^T[i,s] * Wd[i,h]  (lhsT = a^T tile, rhs = Wd)
Batches are processed in pairs (2*S = 512 moving columns) to hit the fp32
moving-operand max and keep the float32r matmul at full rate.
"""

import os
import numpy as np
from contextlib import ExitStack

import concourse.bass as bass
import concourse.mybir as mybir
import concourse.tile as tile
from concourse import bacc
from concourse.bass_utils import run_bass_kernel_spmd

B, S, H, I, E = 32, 256, 1024, 4096, 8
TOP_K = 2
NUM_MOE_LAYERS = 12
NCORES = 8
IS = I // NCORES          # shared-expert intermediate slice per core
HT = H // 128             # h-tiles
P = 128

F32 = mybir.dt.float32
MM_DT = mybir.dt.float32r  # matmul compute dtype (f32r = full-rate, ~tf32)
GELU = mybir.ActivationFunctionType.Gelu_apprx_tanh


def _shared_group(nc, pools, xt_rows, shg, shu, shd, out_rows):
    """Shared-expert MLP (I-slice IS) over a group of 2 pairs (4 batches).

    Weights are resident SBUF tiles:
      shg/shu: [128, HT*IS]  (h-tile t, i-tile i lhsT at cols t*IS + i*128)
      shd:     [128, (IS/128)*H]  (i-tile i rhs at cols i*H)
    """
    xt_p, psgu, pssc, tmp_p, at_p, st_p = (
        pools[k] for k in ("xt", "psgu", "pssc", "tmp", "aT", "stage"))
    S2 = 2 * S
    NI = IS // P
    NP = 2

    xt_t = []
    for pr in range(NP):
        t = xt_p.tile([P, HT * S2], MM_DT, tag="xt", name=f"xts{pr}")
        v = t[:].rearrange("p (t c) -> p t c", c=S2)
        nc.sync.dma_start(
            v[:, :, 0:S],
            xt_rows[2 * pr].rearrange("(t p) s -> p t s", p=P).bitcast(MM_DT))
        nc.scalar.dma_start(
            v[:, :, S:S2],
            xt_rows[2 * pr + 1].rearrange("(t p) s -> p t s", p=P)
            .bitcast(MM_DT))
        xt_t.append(t)

    at_t = at_p.tile([P, NI * NP * S2], MM_DT, tag="aT")
    for i in range(NI):
        ps_g = [psgu.tile([P, S2], F32, tag="ps", name=f"spsg{pr}")
                for pr in range(NP)]
        ps_u = [psgu.tile([P, S2], F32, tag="ps", name=f"spsu{pr}")
                for pr in range(NP)]
        for t in range(HT):
            for pr in range(NP):
                nc.tensor.matmul(ps_g[pr][:],
                                 shg[:, t * IS + i * P: t * IS + (i + 1) * P],
                                 xt_t[pr][:, t * S2:(t + 1) * S2],
                                 start=(t == 0), stop=(t == HT - 1))
        for t in range(HT):
            for pr in range(NP):
                nc.tensor.matmul(ps_u[pr][:],
                                 shu[:, t * IS + i * P: t * IS + (i + 1) * P],
                                 xt_t[pr][:, t * S2:(t + 1) * S2],
                                 start=(t == 0), stop=(t == HT - 1))
        for pr in range(NP):
            tmp_g = tmp_p.tile([P, S2], F32, tag="tmp")
            nc.scalar.activation(tmp_g[:], ps_g[pr][:], GELU)
            col = (i * NP + pr) * S2
            nc.vector.tensor_mul(at_t[:, col:col + S2], tmp_g[:],
                                 ps_u[pr][:])

    for pr in range(NP):
        for ss in range(4):
            st = st_p.tile([P, H], F32, tag="stage")
            for hg in range(2):
                sc = pssc.tile([P, 512], F32, tag="sc")
                for i in range(NI):
                    col = (i * NP + pr) * S2 + ss * P
                    nc.tensor.matmul(sc[:], at_t[:, col:col + P],
                                     shd[:, i * H + hg * 512:
                                         i * H + (hg + 1) * 512],
                                     start=(i == 0), stop=(i == NI - 1))
                nc.vector.tensor_copy(st[:, hg * 512:(hg + 1) * 512], sc[:])
            b = 2 * pr + (ss // 2)
            s0 = (ss % 2) * P
            nc.sync.dma_start(out_rows[b][s0:s0 + P, :], st[:])


CHUNK = 4                 # i-tiles per down-accumulation chunk
GROUPS = ((2, 32), (2, 32), (1, 4), (1, 4))  # (pairs, i-tiles) per slot


def _expert_group(nc, pools, xt_rows, wg_d, wu_d, wd_d, out_rows, n_pairs,
                  ni=I // P):
    """One weight-stream group: n_pairs pairs sharing one expert's weights.

    Interleaved structure: per i-tile, gate/up matmuls for all pairs build
    a^T for a CHUNK of i-tiles; the down projection then contracts that
    chunk into PSUM scratch and accumulates into per-pair SBUF output
    accumulators, so weights stream exactly once per group.

    xt_rows: list of 2*n_pairs DRAM APs [H, S]
    out_rows: list of 2*n_pairs DRAM APs [S, H]
    """
    xt_p, psgu, pssc, tmp_p, at_p, ob_p = (
        pools[k] for k in ("xt", "psgu", "pssc", "tmp", "aT", "outsb"))
    S2 = 2 * S
    NI = ni

    xt_t = []
    for pr in range(n_pairs):
        t = xt_p.tile([P, HT * S2], MM_DT, tag="xt", name=f"xt{pr}")
        v = t[:].rearrange("p (t c) -> p t c", c=S2)
        nc.sync.dma_start(
            v[:, :, 0:S],
            xt_rows[2 * pr].rearrange("(t p) s -> p t s", p=P).bitcast(MM_DT))
        nc.scalar.dma_start(
            v[:, :, S:S2],
            xt_rows[2 * pr + 1].rearrange("(t p) s -> p t s", p=P)
            .bitcast(MM_DT))
        xt_t.append(t)

    # per-pair output accumulators [128 s, (ss, hg) * 512]
    out_sb = [ob_p.tile([P, 4 * 2 * 512], F32, tag="outsb",
                        name=f"osb{pr}") for pr in range(n_pairs)]

    for c0 in range(0, NI, CHUNK):
        chunk = range(c0, min(c0 + CHUNK, NI))
        ch_n = len(chunk)
        at_t = at_p.tile([P, CHUNK * n_pairs * S2], MM_DT, tag="aT")
        wd_ts = []
        for ci, i in enumerate(chunk):
            wg_t = pools["wg"].tile([P, HT * P], MM_DT, tag="wg")
            nc.sync.dma_start(
                wg_t[:].rearrange("p (t i) -> p t i", i=P),
                wg_d.rearrange("(t p) i -> p t i", p=P)
                [:, :, i * P:(i + 1) * P].bitcast(MM_DT))
            wu_t = pools["wu"].tile([P, HT * P], MM_DT, tag="wu")
            nc.scalar.dma_start(
                wu_t[:].rearrange("p (t i) -> p t i", i=P),
                wu_d.rearrange("(t p) i -> p t i", p=P)
                [:, :, i * P:(i + 1) * P].bitcast(MM_DT))
            ps_g = [psgu.tile([P, S2], F32, tag="ps", name=f"psg{pr}")
                    for pr in range(n_pairs)]
            ps_u = [psgu.tile([P, S2], F32, tag="ps", name=f"psu{pr}")
                    for pr in range(n_pairs)]
            for t in range(HT):
                for pr in range(n_pairs):
                    nc.tensor.matmul(ps_g[pr][:], wg_t[:, t * P:(t + 1) * P],
                                     xt_t[pr][:, t * S2:(t + 1) * S2],
                                     start=(t == 0), stop=(t == HT - 1))
            for t in range(HT):
                for pr in range(n_pairs):
                    nc.tensor.matmul(ps_u[pr][:], wu_t[:, t * P:(t + 1) * P],
                                     xt_t[pr][:, t * S2:(t + 1) * S2],
                                     start=(t == 0), stop=(t == HT - 1))
            wd_t = pools["wd"].tile([P, H], MM_DT, tag="wd")
            nc.sync.dma_start(wd_t[:],
                              wd_d[i * P:(i + 1) * P, :].bitcast(MM_DT))
            wd_ts.append(wd_t)
            for pr in range(n_pairs):
                tmp_g = tmp_p.tile([P, S2], F32, tag="tmp")
                nc.scalar.activation(tmp_g[:], ps_g[pr][:], GELU)
                col = (ci * n_pairs + pr) * S2
                nc.vector.tensor_mul(at_t[:, col:col + S2], tmp_g[:],
                                     ps_u[pr][:])

        # down for this chunk: accumulate into out_sb
        for pr in range(n_pairs):
            for ss in range(4):
                for hg in range(2):
                    sc = pssc.tile([P, 512], F32, tag="sc")
                    for ci in range(ch_n):
                        col = (ci * n_pairs + pr) * S2 + ss * P
                        nc.tensor.matmul(sc[:], at_t[:, col:col + P],
                                         wd_ts[ci][:, hg * 512:(hg + 1) * 512],
                                         start=(ci == 0), stop=(ci == ch_n - 1))
                    dst = out_sb[pr][:, (ss * 2 + hg) * 512:
                                     (ss * 2 + hg + 1) * 512]
                    if c0 == 0:
                        nc.vector.tensor_copy(dst, sc[:])
                    else:
                        nc.vector.tensor_add(dst, dst, sc[:])

    for pr in range(n_pairs):
        for ss in range(4):
            b = 2 * pr + (ss // 2)
            s0 = (ss % 2) * P
            nc.sync.dma_start(out_rows[b][s0:s0 + P, :],
                              out_sb[pr][:, ss * H:(ss + 1) * H])


def _build_kernel(C, nreps=1):
    """C = per-core routed-batch capacity (= 2 * sum of slot pairs)."""
    assert C == 2 * sum(np_ for np_, _ in GROUPS)
    nc = bacc.Bacc("TRN2", target_bir_lowering=False, debug=False,
                   num_devices=NCORES)
    xt_r = nc.dram_tensor("xt_r", [C, H, S], F32, kind="ExternalInput").ap()
    xt_all = nc.dram_tensor("xt_all", [B, H, S], F32, kind="ExternalInput").ap()
    wexp = []
    for gi, (np_, ni_) in enumerate(GROUPS):
        wi = ni_ * P
        wexp.append(tuple(
            nc.dram_tensor(f"w{nm}_{gi}", shp, F32, kind="ExternalInput").ap()
            for nm, shp in (("g", [H, wi]), ("u", [H, wi]), ("d", [wi, H]))))
    wg_s = nc.dram_tensor("wg_s", [H, IS], F32, kind="ExternalInput").ap()
    wu_s = nc.dram_tensor("wu_s", [H, IS], F32, kind="ExternalInput").ap()
    wd_s = nc.dram_tensor("wd_s", [IS, H], F32, kind="ExternalInput").ap()
    out_r = nc.dram_tensor("out_r", [C, S, H], F32, kind="ExternalOutput").ap()
    out_s = nc.dram_tensor("out_s", [B, S, H], F32, kind="ExternalOutput").ap()

    with tile.TileContext(nc) as tc, ExitStack() as ctx:
        pools = {
            "xt": ctx.enter_context(tc.tile_pool(name="xt", bufs=2)),
            "psgu": ctx.enter_context(
                tc.tile_pool(name="psgu", bufs=7, space="PSUM")),
            "pssc": ctx.enter_context(
                tc.tile_pool(name="pssc", bufs=1, space="PSUM")),
            "psum": None,  # set below: shared-phase pools alias psgu/pssc
            "tmp": ctx.enter_context(tc.tile_pool(name="tmp", bufs=2)),
            "aT": ctx.enter_context(tc.tile_pool(name="aT", bufs=2)),
            "outsb": ctx.enter_context(tc.tile_pool(name="outsb", bufs=2)),
            "stage": ctx.enter_context(tc.tile_pool(name="stage", bufs=2)),
            "wg": ctx.enter_context(tc.tile_pool(name="wg", bufs=2)),
            "wu": ctx.enter_context(tc.tile_pool(name="wu", bufs=2)),
            "wd": ctx.enter_context(tc.tile_pool(name="wd", bufs=CHUNK)),
            "shw": ctx.enter_context(tc.tile_pool(name="shw", bufs=1)),
        }
        pools["psum"] = pools["psgu"]

        # ---- expert phase: weight-stream groups ---------------------------
        for _rep in range(nreps):
            row = 0
            for gi, (npair, ni_) in enumerate(GROUPS):
                rows = list(range(row, row + 2 * npair))
                _expert_group(nc, pools,
                              [xt_r[r] for r in rows],
                              wexp[gi][0], wexp[gi][1], wexp[gi][2],
                              [out_r[r] for r in rows], npair, ni=ni_)
                row += 2 * npair

        # ---- shared phase: all batches, resident weight slice -------------
        shg = pools["shw"].tile([P, HT * IS], MM_DT, tag="shg")
        shu = pools["shw"].tile([P, HT * IS], MM_DT, tag="shu")
        shd = pools["shw"].tile([P, (IS // P) * H], MM_DT, tag="shd")
        nc.sync.dma_start(
            shg[:].rearrange("p (t c) -> p t c", c=IS),
            wg_s.rearrange("(t p) c -> p t c", p=P).bitcast(MM_DT))
        nc.scalar.dma_start(
            shu[:].rearrange("p (t c) -> p t c", c=IS),
            wu_s.rearrange("(t p) c -> p t c", p=P).bitcast(MM_DT))
        nc.sync.dma_start(
            shd[:].rearrange("p (ib h) -> p ib h", h=H),
            wd_s.rearrange("(ib p) h -> p ib h", p=P).bitcast(MM_DT))

        for _rep in range(nreps):
            for g in range(B // 4):
                rows = list(range(4 * g, 4 * g + 4))
                _shared_group(nc, pools, [xt_all[r] for r in rows],
                              shg[:], shu[:], shd[:],
                              [out_s[r] for r in rows])

    nc.compile()
    return nc


_KERNEL_CACHE = {}


def _get_kernel(groups):
    if groups not in _KERNEL_CACHE:
        global GROUPS
        GROUPS = groups
        _KERNEL_CACHE[groups] = _build_kernel(
            2 * sum(np_ for np_, _ in groups))
    return _KERNEL_CACHE[groups]


def _routing(router_logits):
    """Replicate reference routing in numpy f32: softmax, top-2, renorm."""
    rl = np.asarray(router_logits, np.float32)
    m = rl.max(axis=-1, keepdims=True)
    ex = np.exp(rl - m, dtype=np.float32)
    rw = ex / ex.sum(axis=-1, keepdims=True)
    sel = np.argsort(-rw, axis=-1, kind="stable")[:, :TOP_K]
    w = np.take_along_axis(rw, sel, axis=-1)
    w = w / w.sum(axis=-1, keepdims=True)
    scale = np.float32(1.0 / NUM_MOE_LAYERS)
    w = scale * w + (np.float32(1.0) - scale) * w
    return sel, w.astype(np.float32)


def kernel(x, router_logits, skill_gate, skill_up, skill_down,
           shared_gate, shared_up, shared_down):
    x = np.asarray(x, np.float32)
    skill_gate = np.asarray(skill_gate, np.float32)
    skill_up = np.asarray(skill_up, np.float32)
    skill_down = np.asarray(skill_down, np.float32)
    shared_gate = np.asarray(shared_gate, np.float32)
    shared_up = np.asarray(shared_up, np.float32)
    shared_down = np.asarray(shared_down, np.float32)

    sel, w = _routing(router_logits)
    lists = [[] for _ in range(E)]
    wmap = np.zeros((B, E), np.float32)
    for b in range(B):
        for k in range(TOP_K):
            e = int(sel[b, k])
            lists[e].append(b)
            wmap[b, e] = w[b, k]

    # decompose each expert's routed batches into weight-stream groups of
    # <=2 pairs; entries are (batch, is_real).  Two-pair groups are assigned
    # to one core each ("own" slots); leftover single pairs become
    # tensor-parallel slots split over I across ALL cores.
    groups2, groups1 = [], []
    for e in range(E):
        ent = [(b, True) for b in lists[e]]
        if len(ent) % 2:
            ent.append((0, False))
        pairs = [ent[i:i + 2] for i in range(0, len(ent), 2)]
        for i in range(0, len(pairs) - 1, 2):
            groups2.append((e, pairs[i] + pairs[i + 1]))
        if len(pairs) % 2:
            groups1.append((e, pairs[-1]))
    n2 = max(1, -(-len(groups2) // NCORES))
    n_tp = len(groups1)
    TPI = I // NCORES  # i-columns per core for a tp slot
    cfg = ((2, I // P),) * n2 + ((1, TPI // P),) * n_tp
    dummy2 = (0, [(0, False)] * 4)
    groups2 += [dummy2] * (n2 * NCORES - len(groups2))

    xt = np.ascontiguousarray(x.transpose(0, 2, 1))  # [B, H, S]
    nc = _get_kernel(cfg)

    in_maps = []
    core_slots = []
    for c in range(NCORES):
        own = [groups2[c * n2 + j] for j in range(n2)]
        core_slots.append(own)
        batches = [b for _, ent in own for b, _ in ent]
        batches += [b for _, ent in groups1 for b, _ in ent]
        m = {
            "xt_r": np.ascontiguousarray(xt[batches]),
            "xt_all": xt,
            "wg_s": np.ascontiguousarray(shared_gate[:, c * IS:(c + 1) * IS]),
            "wu_s": np.ascontiguousarray(shared_up[:, c * IS:(c + 1) * IS]),
            "wd_s": np.ascontiguousarray(shared_down[c * IS:(c + 1) * IS, :]),
        }
        for gi, (e, _) in enumerate(own):
            m[f"wg_{gi}"] = skill_gate[e]
            m[f"wu_{gi}"] = skill_up[e]
            m[f"wd_{gi}"] = skill_down[e]
        for tj, (e, _) in enumerate(groups1):
            gi = n2 + tj
            sl = slice(c * TPI, (c + 1) * TPI)
            m[f"wg_{gi}"] = np.ascontiguousarray(skill_gate[e][:, sl])
            m[f"wu_{gi}"] = np.ascontiguousarray(skill_up[e][:, sl])
            m[f"wd_{gi}"] = np.ascontiguousarray(skill_down[e][sl, :])
        in_maps.append(m)

    trace = bool(os.environ.get("TRNK_TRACE"))
    res = run_bass_kernel_spmd(nc, in_maps, core_ids=list(range(NCORES)),
                               trace=trace,
                               trace_cores=list(range(NCORES)) if trace else None)
    kernel.last_exec_time_ns = res.exec_time_ns
    kernel.last_results = res
    kernel.last_nc = nc
    kernel.last_in_maps = in_maps

    out = np.zeros((B, S, H), np.float32)
    n_own_rows = 0
    for c in range(NCORES):
        r = res.results[c]["out_r"]
        row = 0
        for e, ent in core_slots[c]:
            for b, real in ent:
                if real:
                    out[b] += wmap[b, e] * r[row]
                row += 1
        n_own_rows = row
    # tp slots: rows are partial (I-slice) sums — reduce across cores
    for tj, (e, ent) in enumerate(groups1):
        for k, (b, real) in enumerate(ent):
            if real:
                row = n_own_rows + 2 * tj + k
                part = sum(res.results[c]["out_r"][row]
                           for c in range(NCORES))
                out[b] += wmap[b, e] * part
    for c in range(NCORES):
        out += res.results[c]["out_s"]
    return out



# revision 6
# speedup vs baseline: 3.5973x; 3.5973x over previous
"""MoE GemmaMLP (top-2 of 8 experts + shared expert) on 8 trn2 NeuronCores.

Expert-parallel packing (seed-0 load: 16 two-pair single-expert weight-stream
groups, one 2+2 pair config per core, + 2 leftover pairs run tensor-parallel
over I on all cores; shared expert split over I, batch-replicated), bf16
end-to-end:
  - all matmuls bf16 (tolerance 2e-2 >> bf16's ~4.7e-3 measured): halves
    every DMA transfer and the SBUF footprint vs f32/f32r. HW-measured MM
    issue rate at N=512 is ~256 ns/MM for bf16 and f32r alike (slope-timed
    microbench), and LDWEIGHTS is hidden by the PE reorder window (4x weight
    reuse saves <10 ns/MM), so the dtype change buys bandwidth, not FLOPs.
  - down projection is weight-stationary: out^T[h,s] = sum_i wd[i,h]^T aT
    with i as the outer (PSUM-accumulated) loop; outputs land transposed
    [H, S] and the host transposes during the weighted gather for free.
    PSUM drains drop 2x vs activation-stationary.
  - weights stream as per-chunk slabs (1-2KB contiguous runs, ~0.25-1MB per
    DMA) alternating between the SP and ACT HWDGE rings.
  - PSUM: 4 banks gate/up + 4 banks down (the old down path used one bank,
    serializing every drain).
  - xt pool holds 4 slots (2 live + 2 prefetch) so the next group's inputs
    load during the current group's compute; shared-expert slabs are loaded
    after the first expert group's DMAs so the first matmul starts ~15us
    earlier.
Per-core: 4800 matmuls x 256 ns =~ 1.23 ms PE floor; measured steady-state
~1.30 ms (pipelined nreps-slope timing), vs 1.86 ms for the f32r baseline.
"""

import os
import numpy as np
import ml_dtypes

import concourse.mybir as mybir
import concourse.tile as tile
from concourse import bacc
from concourse.bass_utils import run_bass_kernel_spmd

B, S, H, I, E = 32, 256, 1024, 4096, 8
TOP_K = 2
NUM_MOE_LAYERS = 12
NCORES = 8
IS = I // NCORES          # shared-expert intermediate slice per core
HT = H // 128             # h-tiles
P = 128
S2 = 2 * S                # moving columns per pair (2 batches)
CHUNK = 4                 # i-tiles per weight-stream / down-accum chunk

F32 = mybir.dt.float32
BF = mybir.dt.bfloat16
NPBF = ml_dtypes.bfloat16
GELU = mybir.ActivationFunctionType.Gelu_apprx_tanh

GROUPS = ((2, 32), (2, 32), (1, 4), (1, 4))  # (pairs, i-tiles) per slot


def _group(nc, pools, xt_rows, wg_d, wu_d, wd_d, out_rows, n_pairs, ni,
           res_w=None, ring_flip=0):
    """One weight-stream group: n_pairs pairs sharing one expert's weights.

    xt_rows: 2*n_pairs DRAM APs [H, S] (bf16)
    out_rows: 2*n_pairs DRAM APs [H, S] (bf16, transposed output)
    res_w: optional (wg_t, wu_t, wd_t) resident SBUF slabs (shared expert);
        layout identical to the streamed chunk slabs with chn == ni.
    """
    xt_p, psgu, psd_p, tmp_p, at_p = (
        pools[k] for k in ("xt", "psgu", "psd", "tmp", "aT"))
    rings = (nc.sync, nc.scalar)

    xt_t = []
    for pr in range(n_pairs):
        t = xt_p.tile([P, HT * S2], BF, tag="xt", name=f"xt{pr}")
        v = t[:].rearrange("p (t c) -> p t c", c=S2)
        rings[ring_flip].dma_start(
            v[:, :, 0:S], xt_rows[2 * pr].rearrange("(t p) s -> p t s", p=P))
        rings[1 - ring_flip].dma_start(
            v[:, :, S:S2],
            xt_rows[2 * pr + 1].rearrange("(t p) s -> p t s", p=P))
        xt_t.append(t)

    n_chunks = (ni + CHUNK - 1) // CHUNK
    acc = None
    if n_chunks > 1:
        acc = pools["acc"].tile([P, HT * n_pairs * S2], F32, tag="acc")
    stage = pools["stage"].tile([P, HT * n_pairs * S2], BF, tag="stage")
    DHT = 2 if n_pairs == 2 else 4  # down ht-tiles per PSUM pass (4 banks)

    for ci in range(n_chunks):
        c0 = ci * CHUNK
        chn = min(CHUNK, ni - c0)
        if res_w is not None:
            wg_t, wu_t, wd_t = res_w
            wcols, woff = ni * P, c0 * P
        else:
            wcols, woff = chn * P, 0
            wg_t = pools["wg"].tile([P, HT * chn * P], BF, tag="wg")
            rings[(ci + ring_flip) % 2].dma_start(
                wg_t[:].rearrange("p (t c) -> p t c", c=chn * P),
                wg_d.rearrange("(t p) c -> p t c", p=P)
                [:, :, c0 * P:(c0 + chn) * P])
            wu_t = pools["wu"].tile([P, HT * chn * P], BF, tag="wu")
            rings[(ci + 1 + ring_flip) % 2].dma_start(
                wu_t[:].rearrange("p (t c) -> p t c", c=chn * P),
                wu_d.rearrange("(t p) c -> p t c", p=P)
                [:, :, c0 * P:(c0 + chn) * P])
            wd_t = pools["wd"].tile([P, chn * H], BF, tag="wd")
            rings[(ci + ring_flip) % 2].dma_start(
                wd_t[:].rearrange("p (i h) -> p i h", h=H),
                wd_d.rearrange("(i p) h -> p i h", p=P)[:, c0:c0 + chn, :])

        at_t = at_p.tile([P, CHUNK * n_pairs * S2], BF, tag="aT")
        for il in range(chn):
            ps_g = [psgu.tile([P, S2], F32, tag="ps", name=f"psg{pr}")
                    for pr in range(n_pairs)]
            ps_u = [psgu.tile([P, S2], F32, tag="ps", name=f"psu{pr}")
                    for pr in range(n_pairs)]
            for t in range(HT):
                lhs = wg_t[:, t * wcols + woff + il * P:
                           t * wcols + woff + (il + 1) * P]
                for pr in range(n_pairs):
                    nc.tensor.matmul(ps_g[pr][:], lhs,
                                     xt_t[pr][:, t * S2:(t + 1) * S2],
                                     start=(t == 0), stop=(t == HT - 1))
            for t in range(HT):
                lhs = wu_t[:, t * wcols + woff + il * P:
                           t * wcols + woff + (il + 1) * P]
                for pr in range(n_pairs):
                    nc.tensor.matmul(ps_u[pr][:], lhs,
                                     xt_t[pr][:, t * S2:(t + 1) * S2],
                                     start=(t == 0), stop=(t == HT - 1))
            for pr in range(n_pairs):
                tmp_g = tmp_p.tile([P, S2], F32, tag="tmp")
                nc.scalar.activation(tmp_g[:], ps_g[pr][:], GELU)
                nc.vector.tensor_mul(
                    at_t[:, (il * n_pairs + pr) * S2:
                         (il * n_pairs + pr + 1) * S2],
                    tmp_g[:], ps_u[pr][:])

        # down: weight-stationary, i-outer accumulation in PSUM
        first, last = (ci == 0), (ci == n_chunks - 1)
        wdoff = c0 if res_w is not None else 0
        for htp in range(0, HT, DHT):
            psd = [[psd_p.tile([P, S2], F32, tag="psd", name=f"psd{hl}_{pr}")
                    for pr in range(n_pairs)] for hl in range(DHT)]
            for il in range(chn):
                for hl in range(DHT):
                    ht = htp + hl
                    lhs = wd_t[:, (wdoff + il) * H + ht * P:
                               (wdoff + il) * H + (ht + 1) * P]
                    for pr in range(n_pairs):
                        nc.tensor.matmul(
                            psd[hl][pr][:], lhs,
                            at_t[:, (il * n_pairs + pr) * S2:
                                 (il * n_pairs + pr + 1) * S2],
                            start=(il == 0), stop=(il == chn - 1))
            for hl in range(DHT):
                ht = htp + hl
                for pr in range(n_pairs):
                    col = (ht * n_pairs + pr) * S2
                    if first and last:
                        nc.vector.tensor_copy(stage[:, col:col + S2],
                                              psd[hl][pr][:])
                    elif last:
                        nc.vector.tensor_add(stage[:, col:col + S2],
                                             acc[:, col:col + S2],
                                             psd[hl][pr][:])
                    elif first:
                        nc.vector.tensor_copy(acc[:, col:col + S2],
                                              psd[hl][pr][:])
                    else:
                        nc.vector.tensor_add(acc[:, col:col + S2],
                                             acc[:, col:col + S2],
                                             psd[hl][pr][:])

    sv = stage[:].rearrange("p (t q c) -> p t q c", q=n_pairs, c=S2)
    for pr in range(n_pairs):
        for half in range(2):
            b = 2 * pr + half
            rings[(b + ring_flip) % 2].dma_start(
                out_rows[b].rearrange("(t p) s -> p t s", p=P),
                sv[:, :, pr, half * S:(half + 1) * S])


def _build_kernel(C, nreps=1):
    """C = per-core routed-batch capacity (= 2 * sum of slot pairs)."""
    assert C == 2 * sum(np_ for np_, _ in GROUPS)
    nc = bacc.Bacc("TRN2", target_bir_lowering=False, debug=False,
                   num_devices=NCORES)
    xt_r = nc.dram_tensor("xt_r", [C, H, S], BF, kind="ExternalInput").ap()
    xt_all = nc.dram_tensor("xt_all", [B, H, S], BF, kind="ExternalInput").ap()
    wexp = []
    for gi, (np_, ni_) in enumerate(GROUPS):
        wi = ni_ * P
        wexp.append(tuple(
            nc.dram_tensor(f"w{nm}_{gi}", shp, BF, kind="ExternalInput").ap()
            for nm, shp in (("g", [H, wi]), ("u", [H, wi]), ("d", [wi, H]))))
    wg_s = nc.dram_tensor("wg_s", [H, IS], BF, kind="ExternalInput").ap()
    wu_s = nc.dram_tensor("wu_s", [H, IS], BF, kind="ExternalInput").ap()
    wd_s = nc.dram_tensor("wd_s", [IS, H], BF, kind="ExternalInput").ap()
    out_r = nc.dram_tensor("out_r", [C, H, S], BF, kind="ExternalOutput").ap()
    out_s = nc.dram_tensor("out_s", [B, H, S], BF, kind="ExternalOutput").ap()

    with tile.TileContext(nc) as tc:
        import contextlib
        with contextlib.ExitStack() as ctx:
            pools = {
                "xt": ctx.enter_context(tc.tile_pool(name="xt", bufs=4)),
                "psgu": ctx.enter_context(
                    tc.tile_pool(name="psgu", bufs=4, space="PSUM")),
                "psd": ctx.enter_context(
                    tc.tile_pool(name="psd", bufs=4, space="PSUM")),
                "tmp": ctx.enter_context(tc.tile_pool(name="tmp", bufs=3)),
                "aT": ctx.enter_context(tc.tile_pool(name="aT", bufs=2)),
                "acc": ctx.enter_context(tc.tile_pool(name="acc", bufs=1)),
                "stage": ctx.enter_context(tc.tile_pool(name="stage", bufs=1)),
                "wg": ctx.enter_context(tc.tile_pool(name="wg", bufs=2)),
                "wu": ctx.enter_context(tc.tile_pool(name="wu", bufs=2)),
                "wd": ctx.enter_context(tc.tile_pool(name="wd", bufs=3)),
                "shw": ctx.enter_context(tc.tile_pool(name="shw", bufs=1)),
            }

            nsi = IS // P
            shg = shu = shd = None

            def _load_shared_slabs():
                # shared-expert resident slabs (layout == streamed chunk
                # slabs with chn = IS//P); emitted after the first expert
                # group so its xt/weight DMAs go first at kernel start
                nonlocal shg, shu, shd
                shg = pools["shw"].tile([P, HT * IS], BF, tag="shg")
                shu = pools["shw"].tile([P, HT * IS], BF, tag="shu")
                shd = pools["shw"].tile([P, nsi * H], BF, tag="shd")
                nc.sync.dma_start(
                    shg[:].rearrange("p (t c) -> p t c", c=IS),
                    wg_s.rearrange("(t p) c -> p t c", p=P))
                nc.scalar.dma_start(
                    shu[:].rearrange("p (t c) -> p t c", c=IS),
                    wu_s.rearrange("(t p) c -> p t c", p=P))
                nc.sync.dma_start(
                    shd[:].rearrange("p (i h) -> p i h", h=H),
                    wd_s.rearrange("(i p) h -> p i h", p=P))

            for _rep in range(nreps):
                row = 0
                flip = 0
                for gi, (npair, ni_) in enumerate(GROUPS):
                    rows = list(range(row, row + 2 * npair))
                    _group(nc, pools,
                           [xt_r[r] for r in rows],
                           wexp[gi][0], wexp[gi][1], wexp[gi][2],
                           [out_r[r] for r in rows], npair, ni_,
                           ring_flip=flip)
                    row += 2 * npair
                    flip ^= 1
                    if shg is None:
                        _load_shared_slabs()

                for g in range(B // 4):
                    rows = list(range(4 * g, 4 * g + 4))
                    _group(nc, pools, [xt_all[r] for r in rows],
                           None, None, None,
                           [out_s[r] for r in rows], 2, nsi,
                           res_w=(shg, shu, shd), ring_flip=g % 2)

    nc.compile()
    return nc


_KERNEL_CACHE = {}


def _get_kernel(groups):
    if groups not in _KERNEL_CACHE:
        global GROUPS
        GROUPS = groups
        _KERNEL_CACHE[groups] = _build_kernel(
            2 * sum(np_ for np_, _ in groups))
    return _KERNEL_CACHE[groups]


def build_with_nreps(groups, nreps):
    """Timing helper: same kernel structure, body repeated nreps times."""
    global GROUPS
    GROUPS = groups
    return _build_kernel(2 * sum(np_ for np_, _ in groups), nreps=nreps)


def _routing(router_logits):
    """Replicate reference routing in numpy f32: softmax, top-2, renorm."""
    rl = np.asarray(router_logits, np.float32)
    m = rl.max(axis=-1, keepdims=True)
    ex = np.exp(rl - m, dtype=np.float32)
    rw = ex / ex.sum(axis=-1, keepdims=True)
    sel = np.argsort(-rw, axis=-1, kind="stable")[:, :TOP_K]
    w = np.take_along_axis(rw, sel, axis=-1)
    w = w / w.sum(axis=-1, keepdims=True)
    scale = np.float32(1.0 / NUM_MOE_LAYERS)
    w = scale * w + (np.float32(1.0) - scale) * w
    return sel, w.astype(np.float32)


def kernel(x, router_logits, skill_gate, skill_up, skill_down,
           shared_gate, shared_up, shared_down):
    x = np.asarray(x, np.float32)
    skill_gate = np.asarray(skill_gate, NPBF)
    skill_up = np.asarray(skill_up, NPBF)
    skill_down = np.asarray(skill_down, NPBF)
    shared_gate = np.asarray(shared_gate, NPBF)
    shared_up = np.asarray(shared_up, NPBF)
    shared_down = np.asarray(shared_down, NPBF)

    sel, w = _routing(router_logits)
    lists = [[] for _ in range(E)]
    wmap = np.zeros((B, E), np.float32)
    for b in range(B):
        for k in range(TOP_K):
            e = int(sel[b, k])
            lists[e].append(b)
            wmap[b, e] = w[b, k]

    # decompose each expert's routed batches into weight-stream groups of
    # <=2 pairs; entries are (batch, is_real).  Two-pair groups are assigned
    # to one core each ("own" slots); leftover single pairs become
    # tensor-parallel slots split over I across ALL cores.
    groups2, groups1 = [], []
    for e in range(E):
        ent = [(b, True) for b in lists[e]]
        if len(ent) % 2:
            ent.append((0, False))
        pairs = [ent[i:i + 2] for i in range(0, len(ent), 2)]
        for i in range(0, len(pairs) - 1, 2):
            groups2.append((e, pairs[i] + pairs[i + 1]))
        if len(pairs) % 2:
            groups1.append((e, pairs[-1]))
    n2 = max(1, -(-len(groups2) // NCORES))
    n_tp = len(groups1)
    TPI = I // NCORES  # i-columns per core for a tp slot
    cfg = ((2, I // P),) * n2 + ((1, TPI // P),) * n_tp
    dummy2 = (0, [(0, False)] * 4)
    groups2 += [dummy2] * (n2 * NCORES - len(groups2))

    xt = np.ascontiguousarray(
        x.transpose(0, 2, 1)).astype(NPBF)  # [B, H, S] bf16
    nc = _get_kernel(cfg)

    in_maps = []
    core_slots = []
    for c in range(NCORES):
        own = [groups2[c * n2 + j] for j in range(n2)]
        core_slots.append(own)
        batches = [b for _, ent in own for b, _ in ent]
        batches += [b for _, ent in groups1 for b, _ in ent]
        m = {
            "xt_r": np.ascontiguousarray(xt[batches]),
            "xt_all": xt,
            "wg_s": np.ascontiguousarray(shared_gate[:, c * IS:(c + 1) * IS]),
            "wu_s": np.ascontiguousarray(shared_up[:, c * IS:(c + 1) * IS]),
            "wd_s": np.ascontiguousarray(shared_down[c * IS:(c + 1) * IS, :]),
        }
        for gi, (e, _) in enumerate(own):
            m[f"wg_{gi}"] = skill_gate[e]
            m[f"wu_{gi}"] = skill_up[e]
            m[f"wd_{gi}"] = skill_down[e]
        for tj, (e, _) in enumerate(groups1):
            gi = n2 + tj
            sl = slice(c * TPI, (c + 1) * TPI)
            m[f"wg_{gi}"] = np.ascontiguousarray(skill_gate[e][:, sl])
            m[f"wu_{gi}"] = np.ascontiguousarray(skill_up[e][:, sl])
            m[f"wd_{gi}"] = np.ascontiguousarray(skill_down[e][sl, :])
        in_maps.append(m)

    trace = bool(os.environ.get("TRNK_TRACE"))
    res = run_bass_kernel_spmd(nc, in_maps, core_ids=list(range(NCORES)),
                               trace=trace,
                               trace_cores=list(range(NCORES)) if trace else None)
    kernel.last_exec_time_ns = res.exec_time_ns
    kernel.last_results = res
    kernel.last_nc = nc
    kernel.last_in_maps = in_maps
    kernel.last_cfg = cfg

    out = np.zeros((B, S, H), np.float32)
    n_own_rows = 0
    for c in range(NCORES):
        r = res.results[c]["out_r"]  # [C, H, S] bf16
        row = 0
        for e, ent in core_slots[c]:
            for b, real in ent:
                if real:
                    out[b] += wmap[b, e] * r[row].astype(np.float32).T
                row += 1
        n_own_rows = row
    # tp slots: rows are partial (I-slice) sums — reduce across cores
    for tj, (e, ent) in enumerate(groups1):
        for k, (b, real) in enumerate(ent):
            if real:
                row = n_own_rows + 2 * tj + k
                part = sum(res.results[c]["out_r"][row].astype(np.float32)
                           for c in range(NCORES))
                out[b] += wmap[b, e] * part.T
    for c in range(NCORES):
        out += res.results[c]["out_s"].astype(np.float32).transpose(0, 2, 1)
    return out
